# revision 35
# baseline (speedup 1.0000x reference)
"""Cross-attention kernel for 8 Trainium2 NeuronCores.

Sharding: (batch x head-group) -- core c handles batch c//4 and heads
4*(c%4)..4*(c%4)+3 (two head-pairs of 128 features each).  Each core reads
only its batch's x/context (8MB instead of 16MB) and writes a [1024, 2048]
fp16 partial; the host sums the 4 partials per batch and adds the bias.
Compute per core is identical to a pure-Megatron head split.

Dataflow is feature-major ("transposed") end to end:
  xT/ctxT [1024, 2048] -> qT/kT [256(hd), 2048] -> scoresT [j, i]
V is produced directly in [token, feature] layout by swapping the matmul
operands (lhsT = a 128-token block of ctxT, rhs = Wv), so nothing is ever
transposed on any engine.  The softmax denominator falls out of the attn@V
matmul as a 65th output row (ones column appended to V).  Matmul operands
are bf16 (fp32 PSUM accumulate).

Iterations run i-outer / head-pair-inner so the out-projection can contract
over both head-pairs (K=256) into one PSUM accumulation.

The emission order is a manual software pipeline tuned to hide the PE stream
inside the ScalarE exp shadow (1147ns per group vs ~640ns of scores+attn@V):
score matmuls for group g+1 are emitted before attn@V of group g
(double-buffered score PSUM), and all projection / out-projection work is
chopped into ~2-matmul micro-pieces that are deadline-scheduled into the
group loop as PE filler, one piece per group.  All misc-PSUM users live in
one strictly-ordered queue sharing a double-buffered bank pair; multi-piece
projection units hold their PSUM tile across their sub-pieces (at most one
foreign allocation can interleave, so two buffers suffice).  The prologue
(weights + chunk 0) is split across the two HWDGE queues (SP + Activation);
steady-state input chunks ride the Activation queue (fresh destination tiles
never stall the exp stream), latency-sensitive small DMAs ride SP.  The
softmax reciprocal is re-laid out to 128 partitions via a DRAM bounce.
"""

import numpy as np

B, N, D, H, DH = 2, 2048, 1024, 16, 64
SCALE = DH ** -0.5
NT = N                  # tokens per core (one batch)
HDC = 4 * DH            # 256 head-dims per core (4 heads = 2 head-pairs)
NCORES = 8

TOKCHUNK = 512          # projection chunk (4 chunks)
ICHUNK = 512            # query chunk in attention (4 per core)
NJT = N // 128          # 16 j-tiles
KT = D // 128           # 8 contraction tiles for projections
NCH = NT // TOKCHUNK    # 4

_PROGRAM = None


def _build_program():
    from contextlib import ExitStack
    from bisect import insort
    import concourse.mybir as mybir
    import concourse.tile as tile
    from concourse import bacc

    F32 = mybir.dt.float32
    F32R = mybir.dt.float32r
    F16 = mybir.dt.float16
    BF16 = mybir.dt.bfloat16
    AF = mybir.ActivationFunctionType

    nc = bacc.Bacc(None, target_bir_lowering=False)

    xt_e = nc.declare_dram_parameter("xt", [NCH, 128, KT, TOKCHUNK], BF16,
                                     isOutput=False)
    ct_e = nc.declare_dram_parameter("ct", [NCH, 128, KT, TOKCHUNK], BF16,
                                     isOutput=False)
    wq_e = nc.declare_dram_parameter("wq", [D, HDC], BF16, isOutput=False)
    wk_e = nc.declare_dram_parameter("wk", [D, HDC], BF16, isOutput=False)
    wv_e = nc.declare_dram_parameter("wv", [D, HDC], BF16, isOutput=False)
    wo_e = nc.declare_dram_parameter("wo", [HDC, D], BF16, isOutput=False)
    out_e = nc.declare_dram_parameter("out", [D, NT], F16, isOutput=True)
    # last i-chunk's hp=1 contribution ships as the raw attn accumulator
    # (64 v-dims + rowsum row, per head); the host divides and applies the
    # tiny out-projection (kills the 16-matmul + 2MB-DMA tail drain)
    tl_e = nc.declare_dram_parameter("tl", [2, 65, ICHUNK], F16,
                                     isOutput=True)

    wq_v = wq_e[:].rearrange("(t p) m -> p t m", p=128)     # [128, 8, 256]
    wk_v = wk_e[:].rearrange("(t p) m -> p t m", p=128)
    wv_v = wv_e[:].rearrange("(t p) m -> p t m", p=128)
    wo_v = wo_e[:].rearrange("(t p) m -> p t m", p=128)     # [128, 2, 1024]
    out_v = out_e[:].rearrange("(t p) n -> p t n", p=128)   # [128, 8, 2048]

    with tile.TileContext(nc) as tc, ExitStack() as ctx:
        const = ctx.enter_context(tc.tile_pool(name="const", bufs=1))
        wpool = ctx.enter_context(tc.tile_pool(name="wpool", bufs=1))
        xsp = ctx.enter_context(tc.tile_pool(name="xsp", bufs=NCH))
        csp = ctx.enter_context(tc.tile_pool(name="csp", bufs=NCH))
        qkp = ctx.enter_context(tc.tile_pool(name="qkp", bufs=1))
        vsb = ctx.enter_context(tc.tile_pool(name="vsb", bufs=1))
        exp = ctx.enter_context(tc.tile_pool(name="exp", bufs=5))
        nrm = ctx.enter_context(tc.tile_pool(name="nrm", bufs=2))
        obp = ctx.enter_context(tc.tile_pool(name="obp", bufs=4))
        drp = ctx.enter_context(tc.tile_pool(name="drp", bufs=2, space="DRAM"))
        ps_s = ctx.enter_context(tc.tile_pool(name="ps_s", bufs=2, space="PSUM"))
        ps_a = ctx.enter_context(tc.tile_pool(name="ps_a", bufs=1, space="PSUM"))
        ps_m = ctx.enter_context(tc.tile_pool(name="ps_m", bufs=2, space="PSUM"))

        # --- constants ---
        ones32 = const.tile([128, 128], F32, tag="ones32", name="ones32")
        nc.gpsimd.memset(ones32[:], 1.0)
        oneb = const.tile([128, 128], BF16, tag="oneb", name="oneb")
        nc.gpsimd.memset(oneb[:], 1.0)
        zerob = const.tile([128, 128], BF16, tag="zerob", name="zerob")
        nc.gpsimd.memset(zerob[:], 0.0)
        wsc = const.tile([128, 128], F32, tag="wsc", name="wsc")

        # --- weights + chunk-0 input, split across the two HWDGE queues so
        # the prologue DMA runs ~2x faster.  Critical order: k-projection
        # (wk + cs0) unblocks first, then v (wv), then q (wq + xs0). ---
        wq_sb = wpool.tile([128, KT, HDC], BF16, tag="wq_sb", name="wq_sb")
        wk_sb = wpool.tile([128, KT, HDC], BF16, tag="wk_sb", name="wk_sb")
        wv_sb = wpool.tile([128, KT, HDC], BF16, tag="wv_sb", name="wv_sb")
        wo_sb = wpool.tile([128, 2, D], BF16, tag="wo_sb", name="wo_sb")
        xs0 = xsp.tile([128, KT, TOKCHUNK], BF16, tag="xs", name="xs0")
        cs0 = csp.tile([128, KT, TOKCHUNK], BF16, tag="cs", name="cs0")
        # Prologue DMA layout is latency-ordered for the first k/q
        # projection HALVES (feature half 0): weights split by FEATURE half
        # (each projection half then depends on a single queue's piece, not
        # both), cs0/xs0 split into t-quarters matching proj sub-pieces so
        # sub 0 can start as soon as t0-1 land.  Only group-0-critical data
        # rides the prologue: the hp=1 weight halves (wk/wq f1, first used
        # at group 20+) and wo (group ~84) are deferred into the piece
        # queue.  cs1 rides sync right behind wq f0 (k-chunk-1 feeds
        # groups 4-7, which arrive early while iteration 0 is PE-bound).
        HH = HDC // 2
        nc.sync.dma_start(wk_sb[:, :, 0:HH], wk_v[:, :, 0:HH])
        nc.scalar.dma_start(wk_sb[:, :, HH:], wk_v[:, :, HH:])
        for qq in range(4):
            eng = nc.sync if qq < 2 else nc.scalar
            eng.dma_start(cs0[:, 2 * qq: 2 * qq + 2],
                          ct_e[0, :, 2 * qq: 2 * qq + 2])
        nc.sync.dma_start(wq_sb[:, :, 0:HH], wq_v[:, :, 0:HH])
        nc.scalar.dma_start(wq_sb[:, :, HH:], wq_v[:, :, HH:])
        for qq in range(4):
            eng = nc.sync if qq < 2 else nc.scalar
            eng.dma_start(xs0[:, 2 * qq: 2 * qq + 2],
                          xt_e[0, :, 2 * qq: 2 * qq + 2])
        nc.sync.dma_start(wv_sb[:, :, 0:HH], wv_v[:, :, 0:HH])
        nc.scalar.dma_start(wv_sb[:, :, HH:], wv_v[:, :, HH:])
        nc.scalar.dma_start(wo_sb[:], wo_v)

        # --- PE warm-up: dummy matmuls while the prologue DMAs stream, so
        # the HAM clock-gate ramps before the first real projection matmuls
        # (which sit on the first-exp critical path).  The two HWDGE queues
        # share the 16 DMA engines (~300GB/s aggregate), so the ~2.5MB of
        # group-0-critical data lands at ~15-16us; the warm-up must span
        # until then or the idle gap resets the PE p-state and the whole
        # prologue projection chain runs at 0.65-1.2GHz. ---
        pw = ps_m.tile([128, 128], F32, tag="pm", name="warm_ps")
        # moving operand is all-zero: the ramp only needs the PE busy,
        # and zero products minimize array toggling so the warm-up burst
        # doesn't trip the HAM power throttle right as real work starts
        for r in range(48):
            nc.tensor.matmul(pw[:], oneb[:], zerob[:],
                             start=(r == 0), stop=(r == 47))
        nc.vector.tensor_copy(wsc[:], pw[:])

        # --- persistent activations ---
        qT_sb = qkp.tile([128, 2, NT], BF16, tag="qT_sb", name="qT_sb")
        kT_sb = qkp.tile([128, 2, NT], BF16, tag="kT_sb", name="kT_sb")
        # V^T for all 4 heads in one tile: [token, head, jtile*(64 dims+one)]
        v_all = vsb.tile([128, 4, NJT * 65], BF16, tag="v_all", name="v_all")
        ones_v = v_all[:].rearrange("p h (j c) -> p (h j) c", c=65)[:, :, 64]
        nc.vector.tensor_copy(ones_v, ones32[:, 0:4 * NJT])

        # ---------- projection chunk emission, chopped into ~2-matmul
        # micro-pieces that fit the per-group exp-shadow filler budget.
        # Multi-sub units hold their misc-PSUM tile across sub-pieces.
        def chunk_pieces(c):
            state = {}

            def dma_cs():
                if c == 0:
                    state["cs"] = cs0
                    return
                cs = csp.tile([128, KT, TOKCHUNK], BF16, tag="cs", name=f"cs{c}")
                # cs is on the critical path of iteration 0 (k/v for all j
                # tiles); alternate queues so no single queue serializes it
                (nc.scalar if c == 2 else nc.sync).dma_start(cs[:], ct_e[c])
                state["cs"] = cs

            def dma_xs():
                if c == 0:
                    state["xs"] = xs0
                    return
                xs = xsp.tile([128, KT, TOKCHUNK], BF16, tag="xs", name=f"xs{c}")
                (nc.sync if c == 2 else nc.scalar).dma_start(xs[:], xt_e[c])
                state["xs"] = xs

            def proj(kind, half, sub, nsub=4):
                """Sub-piece of one q/k projection half: 2 accumulating
                N=512 matmuls; evacuate on the last sub."""
                hsl = slice(128 * half, 128 * (half + 1))
                w, src, dst = {
                    "q": (wq_sb, "xs", qT_sb),
                    "k": (wk_sb, "cs", kT_sb),
                }[kind]
                key = ("p", kind, half)
                if sub == 0:
                    state[key] = ps_m.tile([128, TOKCHUNK], F32, tag="pm",
                                           name=f"p{kind}{c}_{half}")
                p = state[key]
                tpu = KT // nsub
                for t in range(tpu * sub, tpu * (sub + 1)):
                    nc.tensor.matmul(p[:], w[:, t, hsl], state[src][:, t, :],
                                     start=(t == 0), stop=(t == KT - 1))
                if sub == nsub - 1:
                    gsl = slice(c * TOKCHUNK, (c + 1) * TOKCHUNK)
                    nc.vector.tensor_copy(dst[:, half, gsl], p[:])

            def vproj(jj, sub, nsub=2):
                """Sub-piece of the direct-transposed V projection for one
                128-token block: lhsT = ctxT block, rhs = Wv, so the output
                lands as [token, feature] with no transpose anywhere."""
                jt = c * 4 + jj
                key = ("v", jj)
                if sub == 0:
                    state[key] = ps_m.tile([128, TOKCHUNK], F32, tag="pm",
                                           name=f"pv{c}_{jj}")
                p = state[key]
                tpu = KT // nsub
                bsl = slice(128 * jj, 128 * (jj + 1))
                for t in range(tpu * sub, tpu * (sub + 1)):
                    nc.tensor.matmul(p[0:128, 0:HDC],
                                     state["cs"][:, t, bsl], wv_sb[:, t, :],
                                     start=(t == 0), stop=(t == KT - 1))
                if sub == nsub - 1:
                    dst = (v_all[:].rearrange("p h (j c) -> p h j c", c=65)
                           [:, :, jt, 0:64])
                    nc.vector.tensor_copy(
                        dst, p[:, 0:HDC].rearrange("p (h d) -> p h d", d=64))

            # (deadline_group, closure): deadline = global group index by
            # which the piece must be EMITTED (program order = dependencies).
            # chunk c: k half hh feeds iteration (i=0, hp=hh) groups 16hh+4c;
            # v block jj feeds group 4c+jj of iteration 0; q half hh feeds
            # iteration (i=c, hp=hh) = group 16*(2c+hh).  Sub-piece deadlines
            # are SPREAD across the preceding groups so no single group ever
            # pops a whole unit as a burst (which would stall the exp
            # stream); DMAs are issued as early as possible (the queues
            # stream in the background, destination buffers are dedicated).
            # all subs of one unit share a deadline so they stay ADJACENT in
            # the queue (front-pop discipline then guarantees nothing can
            # allocate misc-PSUM between a unit's held sub-pieces); the
            # lookahead boost in the group loop drains them one per group
            # ahead of the deadline instead of bursting at it
            # cs DMAs issue first (groups 0-2); xs DMAs are deferred so
            # they queue BEHIND all cs chunks on both queues (q-units only
            # read xs much later)
            pieces = [(max(0, c - 1), dma_cs),
                      (0 if c == 0 else 4 + c, dma_xs)]
            for half in range(2):
                kdl = 4 * c if half == 0 else 20 + 16 * c
                for sub in range(4):
                    pieces.append((kdl,
                                   lambda hh=half, s=sub: proj("k", hh, s)))
            for jj in range(4):
                # attn@V pops DEPTH groups late, so block jj is not read
                # before group 4c+jj+1; the +1 spreads iter-0's v work
                for sub in range(2):
                    pieces.append((min(4 * c + jj + 1, 15),
                                   lambda j=jj, s=sub: vproj(j, s)))
            for half in range(2):
                qdl = 16 * c if half == 0 else 56 + 16 * c
                for sub in range(4):
                    pieces.append((qdl,
                                   lambda hh=half, s=sub: proj("q", hh, s)))
            return pieces

        # ---------- out-projection pieces for one finished i-chunk
        # (contract K=256 over both head-pairs' on-tiles; hp1 None for the
        # final i-chunk, whose hp=1 part ships via the tail)
        def outproj_pieces(i, on0, on1, last):
            isl = slice(i * ICHUNK, (i + 1) * ICHUNK)

            def one(d8):
                po = ps_m.tile([128, ICHUNK], F32, tag="pm",
                               name=f"po{i}_{d8}")
                dsl = slice(d8 * 128, (d8 + 1) * 128)
                if on1 is not None:
                    nc.tensor.matmul(po[:], wo_sb[:, 0, dsl], on0[:],
                                     start=True, stop=False)
                    nc.tensor.matmul(po[:], wo_sb[:, 1, dsl], on1[:],
                                     start=False, stop=True)
                else:
                    nc.tensor.matmul(po[:], wo_sb[:, 0, dsl], on0[:],
                                     start=True, stop=True)
                ob = obp.tile([128, ICHUNK], F16, tag="ob", name=f"ob{i}_{d8}",
                               bufs=6)
                nc.vector.tensor_copy(ob[:], po[:])
                # bulk output rides the SWDGE queue so the latency-critical
                # small normalize DMAs on the SP queue never sit behind
                # 128KB writes; only i==2 (popped in iteration 7, after the
                # last normalize chain has issued) goes on SP so its writes
                # don't drain past the new tail on the slower SWDGE
                eng = nc.sync if (last or i == 2) else nc.gpsimd
                eng.dma_start(out_v[:, d8, isl], ob[:])

            return [lambda d=d8: one(d) for d8 in range(8)]

        # ---------- emission schedule ----------
        # One strictly-ordered queue of (deadline, seq, closure) for ALL
        # misc-PSUM work; stable deadline order means a multi-sub unit's
        # pieces stay adjacent, so at most one foreign piece (inserted at a
        # later deadline) sits between consecutive allocations -> the 2-buf
        # misc pool can never clobber a held accumulation.
        cq = []
        seq = [0]

        def push(dl, fn):
            insort(cq, (dl, seq[0], fn))
            seq[0] += 1

        for c in range(NCH):
            for dl, fn in chunk_pieces(c):
                push(dl, fn)

        from collections import deque
        # hp-major iteration order: all hp=0 i-chunks first, then hp=1.
        # k-half1 / q-half1 construction then spreads across the
        # underloaded middle iterations instead of clustering at every
        # hp boundary (the hp=0 iterations only need half the k/q work).
        iters = [(i, hp) for hp in range(2) for i in range(N // ICHUNK)]
        n_deadline0 = sum(1 for dl, _, _ in cq if dl == 0)
        # attn@V consumption runs DEPTH groups behind its scores/exp so the
        # PE never sits at an attn@V whose exp hasn't finished (the ex pool's
        # 4 buffers make depth 3 safe: ex[g] is overwritten by scores g+4,
        # and attn@V g is consumed at g+3).  Depth 3 also gives the acc-
        # recycle copies at iteration boundaries a full group of slack.
        DEPTH = 4
        pend = deque()        # emitted-but-unconsumed attn@V closures
        cq_rate = [(len(cq) - n_deadline0) / 112.0]
        cq_credit = [0.0]
        on_tiles = {}

        for it, (i, hp) in enumerate(iters):
            isl = slice(i * ICHUNK, (i + 1) * ICHUNK)
            acc = ps_a.tile([128, 2 * ICHUNK], F32, tag="acc",
                            name=f"acc{i}_{hp}")

            def scores(hp, i, j, isl):
                ss = ps_s.tile([128, 2 * 512], F32, tag="ss",
                               name=f"ss{i}_{hp}_{j}")
                jsl = slice(j * 128, (j + 1) * 128)
                for h in range(2):
                    hs = slice(64 * h, 64 * h + 64)
                    nc.tensor.matmul(ss[:, 512 * h: 512 * (h + 1)],
                                     kT_sb[hs, hp, jsl], qT_sb[hs, hp, isl],
                                     start=True, stop=True)
                ex = exp.tile([128, 2 * 512], BF16, tag="ex",
                              name=f"ex{i}_{hp}_{j}")
                nc.scalar.activation(ex[:], ss[:], AF.Exp)
                return ex

            def attnv(hp, j, ex, acc):
                for h in range(2):
                    nc.tensor.matmul(
                        acc[0:65, ICHUNK * h: ICHUNK * (h + 1)],
                        v_all[:, 2 * hp + h, 65 * j: 65 * j + 65],
                        ex[:, 512 * h: 512 * (h + 1)],
                        start=(j == 0), stop=(j == NJT - 1))

            for g in range(NJT):
                gg = it * NJT + g
                # correctness: pieces this group's scores/attnv read must be
                # emitted first (program order defines dependencies)
                while cq and cq[0][0] <= gg:
                    cq.pop(0)[2]()
                # consume the attn@V from DEPTH groups ago (its normalize
                # closure, if any, is composed onto it) BEFORE this group's
                # scores: the scores matmul waits on the exp(g-2) bank
                # recycle, and parking the PE on that semaphore with no
                # work in between resets the p-state ramp every group
                if len(pend) >= DEPTH:
                    pend.popleft()()
                ex = scores(hp, i, g, isl)
                pend.append(lambda hh=hp, gg_=g, e=ex, a=acc:
                            attnv(hh, gg_, e, a))
                if it == 4 and g == DEPTH:
                    # last i-chunk's hp=0 contribution: its on-tile became
                    # ready at this iteration's g=0 flush (hp-major order),
                    # and iterations 4-5 have spare filler capacity while
                    # iteration 7 is loaded with outproj(2) already.  +3
                    # keeps the first piece clear of the normalize's DRAM
                    # round-trip (~3us) so the PE never parks on it.
                    for k, fn in enumerate(outproj_pieces(3, on_tiles[(3, 0)],
                                                          None, True)):
                        push(gg + 3 + 2 * k, fn)
                # paced fillers: ~1 micro-piece per group; when a deadline
                # looms within 8 groups, guarantee at least one pop so a
                # unit's subs drain gradually instead of bursting at the
                # deadline (which would stall the exp stream)
                cq_credit[0] += cq_rate[0]
                if cq and cq[0][0] <= gg + 8:
                    cq_credit[0] = max(cq_credit[0], 1.0)
                while cq and cq_credit[0] >= 1.0:
                    cq_credit[0] -= 1.0
                    cq.pop(0)[2]()
            if it == len(iters) - 1:
                while pend:
                    pend.popleft()()
                # ---- tail: ship the raw hp=1 accumulator (unnormalized
                # per-head products + rowsum row) for the last i-chunk;
                # host divides and applies the small out-projection.  Two
                # casts (DVE + the now-idle ScalarE) + two small DMAs.
                tlA = nrm.tile([128, ICHUNK], F16, tag="tlA", name="tlA")
                tlB = nrm.tile([128, ICHUNK], F16, tag="tlB", name="tlB")
                nc.vector.tensor_copy(tlA[0:65, :], acc[0:65, 0:ICHUNK])
                nc.scalar.copy(tlB[0:65, :], acc[0:65, ICHUNK:])
                nc.sync.dma_start(tl_e[0], tlA[0:65, :])
                nc.scalar.dma_start(tl_e[1], tlB[0:65, :])
                continue

            # ---- normalization: deferred into the next iteration's group-0
            # slot so this iteration's last attn@V flows straight into the
            # next one's scores with filler in between ----
            def make_norm(acc=acc, i=i, hp=hp, it=it):
                def norm():
                    accs = nrm.tile([128, 2 * ICHUNK], F32R, tag="accs",
                                    name=f"accs{i}_{hp}")
                    with nc.allow_low_precision(
                            reason="attn out + softmax denom fp32r"):
                        # h0 on DVE, h1 on the briefly-idle ScalarE so
                        # acc's PSUM banks recycle within one group period
                        nc.vector.tensor_copy(accs[0:65, 0:ICHUNK],
                                              acc[0:65, 0:ICHUNK])
                        nc.scalar.copy(accs[0:65, ICHUNK:],
                                       acc[0:65, ICHUNK:])
                    # rowsum [1, 1024]@p64 -> DRAM -> [128, 8] -> recip ->
                    # DRAM -> broadcast-DMA to 128 partitions (no PE/PSUM).
                    # The relayout bounce is essential: DVE RECIPROCAL is
                    # ~6.4ns/elem PER PARTITION (6.5us on [1, 1024] vs 203ns
                    # on [128, 8]).
                    rs_d = drp.tile([2 * ICHUNK], F32R, tag="rs_d",
                                    name=f"rs_d{i}_{hp}")
                    nc.sync.dma_start(rs_d[:], accs[64:65, :])
                    rs128 = nrm.tile([128, 8], F32R, tag="rs128",
                                     name=f"rs128{i}_{hp}")
                    nc.sync.dma_start(
                        rs128[:], rs_d[:].rearrange("(p a) -> p a", p=128))
                    rr128 = nrm.tile([128, 8], F32R, tag="rr128",
                                     name=f"rr128{i}_{hp}")
                    with nc.allow_low_precision(
                            reason="softmax denom recip fp32r"):
                        nc.vector.reciprocal(rr128[:], rs128[:])
                    rr_d = drp.tile([2 * ICHUNK], F32R, tag="rr_d",
                                    name=f"rr_d{i}_{hp}")
                    nc.sync.dma_start(
                        rr_d[:].rearrange("(p a) -> p a", p=128), rr128[:])
                    bcs = nrm.tile([128, 2 * ICHUNK], F32R, tag="bcs",
                                   name=f"bcs{i}_{hp}")
                    nc.sync.dma_start(
                        bcs[:],
                        rr_d[:].unsqueeze(0).broadcast_to([128, 2 * ICHUNK]))
                    on = nrm.tile([128, ICHUNK], BF16, tag="on",
                                  name=f"on{i}_{hp}", bufs=6)
                    with nc.allow_low_precision(
                            reason="attn out normalize bf16"):
                        for h in range(2):
                            nc.vector.tensor_mul(
                                on[64 * h: 64 * h + 64, :],
                                accs[0:64, ICHUNK * h: ICHUNK * (h + 1)],
                                bcs[0:64, ICHUNK * h: ICHUNK * (h + 1)])
                    on_tiles[(i, hp)] = on
                    if hp == 1:
                        # queue i's out-projection into upcoming filler
                        # slots (8 pieces, due progressively).  First piece
                        # +4 groups after the boundary so the normalize's
                        # DRAM round-trip (~3us) completes before the PE
                        # reaches the first outproj matmul in program order.
                        for k, fn in enumerate(
                                outproj_pieces(i, on_tiles[(i, 0)], on,
                                               False)):
                            push((it + 1) * NJT + 4 + 2 * k, fn)
                return norm
            # ride the normalize right behind this iteration's last attn@V
            last_attnv = pend.pop()
            pend.append(lambda f=last_attnv, n=make_norm(): (f(), n()))

        while cq:
            cq.pop(0)[2]()

    nc.compile()
    return nc


def _get_program():
    global _PROGRAM
    if _PROGRAM is None:
        _PROGRAM = _build_program()
    return _PROGRAM


def _prepare_in_maps(x, context, Wq, Wk, Wv, Wo, bo):
    import ml_dtypes
    bf16 = ml_dtypes.bfloat16

    x = np.asarray(x, dtype=np.float32)
    context = np.asarray(context, dtype=np.float32)
    Wq = np.asarray(Wq, dtype=np.float32)
    Wk = np.asarray(Wk, dtype=np.float32)
    Wv = np.asarray(Wv, dtype=np.float32)
    Wo = np.asarray(Wo, dtype=np.float32)
    Wk_s = Wk * np.float32(SCALE)

    def featmaj(a):  # [N, D] -> [NCH, 128, KT, TOKCHUNK] bf16
        t = a.T.reshape(KT, 128, NCH, TOKCHUNK).transpose(2, 1, 0, 3)
        return np.ascontiguousarray(t).astype(bf16)

    xT = [featmaj(x[b]) for b in range(B)]
    cT = [featmaj(context[b]) for b in range(B)]

    in_maps = []
    for c in range(NCORES):
        b, g = c // 4, c % 4
        cs = slice(g * HDC, (g + 1) * HDC)
        in_maps.append({
            "xt": xT[b],
            "ct": cT[b],
            "wq": np.ascontiguousarray(Wq[:, cs]).astype(bf16),
            "wk": np.ascontiguousarray(Wk_s[:, cs]).astype(bf16),
            "wv": np.ascontiguousarray(Wv[:, cs]).astype(bf16),
            "wo": np.ascontiguousarray(Wo[cs, :]).astype(bf16),
        })
    return in_maps


def _gather(results, bo, Wo):
    bo = np.asarray(bo, dtype=np.float32)
    Wo = np.asarray(Wo, dtype=np.float32)
    last = slice(NT - ICHUNK, NT)
    outs = []
    for b in range(B):
        acc = results[4 * b]["out"].astype(np.float64)
        for g in range(1, 4):
            acc += results[4 * b + g]["out"]
        for g in range(4):
            r = results[4 * b + g]
            tl = r["tl"].astype(np.float64)          # [2, 65, ICHUNK]
            for h in range(2):
                attn = tl[h, 0:64] / tl[h, 64]       # [64, ICHUNK]
                row0 = g * HDC + 128 + 64 * h
                wo_h = Wo[row0: row0 + 64]           # [64, D]
                acc[:, last] += wo_h.T @ attn
        outs.append(acc.T.astype(np.float32) + bo)
    return np.stack(outs).reshape(B, N, D)


def kernel(x, context, Wq, Wk, Wv, Wo, bo):
    from concourse.bass_utils import run_bass_kernel_spmd

    in_maps = _prepare_in_maps(x, context, Wq, Wk, Wv, Wo, bo)
    nc = _get_program()
    res = run_bass_kernel_spmd(nc, in_maps, list(range(NCORES)))
    return _gather(res.results, bo, Wo)



# revision 36
# speedup vs baseline: 1.0080x; 1.0080x over previous
"""Cross-attention kernel for 8 Trainium2 NeuronCores.

Sharding: (batch x head-group) -- core c handles batch c//4 and heads
4*(c%4)..4*(c%4)+3 (two head-pairs of 128 features each).  Each core reads
only its batch's x/context (8MB instead of 16MB) and writes a [1024, 2048]
fp16 partial; the host sums the 4 partials per batch and adds the bias.
Compute per core is identical to a pure-Megatron head split.

Dataflow is feature-major ("transposed") end to end:
  xT/ctxT [1024, 2048] -> qT/kT [256(hd), 2048] -> scoresT [j, i]
V is produced directly in [token, feature] layout by swapping the matmul
operands (lhsT = a 128-token block of ctxT, rhs = Wv), so nothing is ever
transposed on any engine.  The softmax denominator falls out of the attn@V
matmul as a 65th output row (ones column appended to V).  Matmul operands
are bf16 (fp32 PSUM accumulate).

Iterations run i-outer / head-pair-inner so the out-projection can contract
over both head-pairs (K=256) into one PSUM accumulation.

The emission order is a manual software pipeline tuned to hide the PE stream
inside the ScalarE exp shadow (1147ns per group vs ~640ns of scores+attn@V):
score matmuls for group g+1 are emitted before attn@V of group g
(double-buffered score PSUM), and all projection / out-projection work is
chopped into ~2-matmul micro-pieces that are deadline-scheduled into the
group loop as PE filler, one piece per group.  All misc-PSUM users live in
one strictly-ordered queue sharing a double-buffered bank pair; multi-piece
projection units hold their PSUM tile across their sub-pieces (at most one
foreign allocation can interleave, so two buffers suffice).  The prologue
(weights + chunk 0) is split across the two HWDGE queues (SP + Activation);
steady-state input chunks ride the Activation queue (fresh destination tiles
never stall the exp stream), latency-sensitive small DMAs ride SP.  The
softmax reciprocal is re-laid out to 128 partitions via a DRAM bounce.
"""

import numpy as np

B, N, D, H, DH = 2, 2048, 1024, 16, 64
SCALE = DH ** -0.5
NT = N                  # tokens per core (one batch)
HDC = 4 * DH            # 256 head-dims per core (4 heads = 2 head-pairs)
NCORES = 8

TOKCHUNK = 512          # projection chunk (4 chunks)
ICHUNK = 512            # query chunk in attention (4 per core)
NJT = N // 128          # 16 j-tiles
KT = D // 128           # 8 contraction tiles for projections
NCH = NT // TOKCHUNK    # 4

_PROGRAM = None


def _build_program():
    from contextlib import ExitStack
    from bisect import insort
    import concourse.mybir as mybir
    import concourse.tile as tile
    from concourse import bacc

    F32 = mybir.dt.float32
    F32R = mybir.dt.float32r
    F16 = mybir.dt.float16
    BF16 = mybir.dt.bfloat16
    AF = mybir.ActivationFunctionType

    nc = bacc.Bacc(None, target_bir_lowering=False)

    xt_e = nc.declare_dram_parameter("xt", [NCH, 128, KT, TOKCHUNK], BF16,
                                     isOutput=False)
    ct_e = nc.declare_dram_parameter("ct", [NCH, 128, KT, TOKCHUNK], BF16,
                                     isOutput=False)
    wq_e = nc.declare_dram_parameter("wq", [D, HDC], BF16, isOutput=False)
    wk_e = nc.declare_dram_parameter("wk", [D, HDC], BF16, isOutput=False)
    wv_e = nc.declare_dram_parameter("wv", [D, HDC], BF16, isOutput=False)
    wo_e = nc.declare_dram_parameter("wo", [HDC, D], BF16, isOutput=False)
    out_e = nc.declare_dram_parameter("out", [D, NT], F16, isOutput=True)
    # last i-chunk's hp=1 contribution ships as the raw attn accumulator
    # (64 v-dims + rowsum row, per head); the host divides and applies the
    # tiny out-projection (kills the 16-matmul + 2MB-DMA tail drain)
    tl_e = nc.declare_dram_parameter("tl", [2, 65, ICHUNK], F16,
                                     isOutput=True)

    wq_v = wq_e[:].rearrange("(t p) m -> p t m", p=128)     # [128, 8, 256]
    wk_v = wk_e[:].rearrange("(t p) m -> p t m", p=128)
    wv_v = wv_e[:].rearrange("(t p) m -> p t m", p=128)
    wo_v = wo_e[:].rearrange("(t p) m -> p t m", p=128)     # [128, 2, 1024]
    out_v = out_e[:].rearrange("(t p) n -> p t n", p=128)   # [128, 8, 2048]

    with tile.TileContext(nc) as tc, ExitStack() as ctx:
        const = ctx.enter_context(tc.tile_pool(name="const", bufs=1))
        wpool = ctx.enter_context(tc.tile_pool(name="wpool", bufs=1))
        xsp = ctx.enter_context(tc.tile_pool(name="xsp", bufs=NCH))
        csp = ctx.enter_context(tc.tile_pool(name="csp", bufs=NCH))
        qkp = ctx.enter_context(tc.tile_pool(name="qkp", bufs=1))
        vsb = ctx.enter_context(tc.tile_pool(name="vsb", bufs=1))
        exp = ctx.enter_context(tc.tile_pool(name="exp", bufs=5))
        nrm = ctx.enter_context(tc.tile_pool(name="nrm", bufs=2))
        obp = ctx.enter_context(tc.tile_pool(name="obp", bufs=4))
        drp = ctx.enter_context(tc.tile_pool(name="drp", bufs=2, space="DRAM"))
        ps_s = ctx.enter_context(tc.tile_pool(name="ps_s", bufs=2, space="PSUM"))
        ps_a = ctx.enter_context(tc.tile_pool(name="ps_a", bufs=1, space="PSUM"))
        ps_m = ctx.enter_context(tc.tile_pool(name="ps_m", bufs=2, space="PSUM"))

        # --- constants ---
        ones32 = const.tile([128, 128], F32, tag="ones32", name="ones32")
        nc.gpsimd.memset(ones32[:], 1.0)
        oneb = const.tile([128, 128], BF16, tag="oneb", name="oneb")
        nc.gpsimd.memset(oneb[:], 1.0)
        zerob = const.tile([128, 128], BF16, tag="zerob", name="zerob")
        nc.gpsimd.memset(zerob[:], 0.0)
        wsc = const.tile([128, 128], F32, tag="wsc", name="wsc")

        # --- weights + chunk-0 input, split across the two HWDGE queues so
        # the prologue DMA runs ~2x faster.  Critical order: k-projection
        # (wk + cs0) unblocks first, then v (wv), then q (wq + xs0). ---
        wq_sb = wpool.tile([128, KT, HDC], BF16, tag="wq_sb", name="wq_sb")
        wk_sb = wpool.tile([128, KT, HDC], BF16, tag="wk_sb", name="wk_sb")
        wv_sb = wpool.tile([128, KT, HDC], BF16, tag="wv_sb", name="wv_sb")
        wo_sb = wpool.tile([128, 2, D], BF16, tag="wo_sb", name="wo_sb")
        xs0 = xsp.tile([128, KT, TOKCHUNK], BF16, tag="xs", name="xs0")
        cs0 = csp.tile([128, KT, TOKCHUNK], BF16, tag="cs", name="cs0")
        # Prologue DMA layout is latency-ordered for the first k/q
        # projection HALVES (feature half 0): weights split by FEATURE half
        # (each projection half then depends on a single queue's piece, not
        # both), cs0/xs0 split into t-quarters matching proj sub-pieces so
        # sub 0 can start as soon as t0-1 land.  Only group-0-critical data
        # rides the prologue: the hp=1 weight halves (wk/wq f1, first used
        # at group 20+) and wo (group ~84) are deferred into the piece
        # queue.  cs1 rides sync right behind wq f0 (k-chunk-1 feeds
        # groups 4-7, which arrive early while iteration 0 is PE-bound).
        HH = HDC // 2
        nc.sync.dma_start(wk_sb[:, :, 0:HH], wk_v[:, :, 0:HH])
        nc.scalar.dma_start(wk_sb[:, :, HH:], wk_v[:, :, HH:])
        for qq in range(4):
            eng = nc.sync if qq < 2 else nc.scalar
            eng.dma_start(cs0[:, 2 * qq: 2 * qq + 2],
                          ct_e[0, :, 2 * qq: 2 * qq + 2])
        nc.sync.dma_start(wq_sb[:, :, 0:HH], wq_v[:, :, 0:HH])
        nc.scalar.dma_start(wq_sb[:, :, HH:], wq_v[:, :, HH:])
        for qq in range(4):
            eng = nc.sync if qq < 2 else nc.scalar
            eng.dma_start(xs0[:, 2 * qq: 2 * qq + 2],
                          xt_e[0, :, 2 * qq: 2 * qq + 2])
        nc.sync.dma_start(wv_sb[:, :, 0:HH], wv_v[:, :, 0:HH])
        nc.scalar.dma_start(wv_sb[:, :, HH:], wv_v[:, :, HH:])
        nc.scalar.dma_start(wo_sb[:], wo_v)

        # --- PE warm-up: dummy matmuls while the prologue DMAs stream, so
        # the HAM clock-gate ramps before the first real projection matmuls
        # (which sit on the first-exp critical path).  The two HWDGE queues
        # share the 16 DMA engines (~300GB/s aggregate), so the ~2.5MB of
        # group-0-critical data lands at ~15-16us; the warm-up must span
        # until then or the idle gap resets the PE p-state and the whole
        # prologue projection chain runs at 0.65-1.2GHz. ---
        pw = ps_m.tile([128, 128], F32, tag="pm", name="warm_ps")
        # moving operand is all-zero: the ramp only needs the PE busy,
        # and zero products minimize array toggling so the warm-up burst
        # doesn't trip the HAM power throttle right as real work starts
        for r in range(48):
            nc.tensor.matmul(pw[:], oneb[:], zerob[:],
                             start=(r == 0), stop=(r == 47))
        nc.vector.tensor_copy(wsc[:], pw[:])

        # --- persistent activations ---
        qT_sb = qkp.tile([128, 2, NT], BF16, tag="qT_sb", name="qT_sb")
        kT_sb = qkp.tile([128, 2, NT], BF16, tag="kT_sb", name="kT_sb")
        # V^T for all 4 heads in one tile: [token, head, jtile*(64 dims+one)]
        v_all = vsb.tile([128, 4, NJT * 65], BF16, tag="v_all", name="v_all")
        ones_v = v_all[:].rearrange("p h (j c) -> p (h j) c", c=65)[:, :, 64]
        nc.vector.tensor_copy(ones_v, ones32[:, 0:4 * NJT])

        # ---------- projection chunk emission, chopped into ~2-matmul
        # micro-pieces that fit the per-group exp-shadow filler budget.
        # Multi-sub units hold their misc-PSUM tile across sub-pieces.
        def chunk_pieces(c):
            state = {}

            def dma_cs():
                if c == 0:
                    state["cs"] = cs0
                    return
                cs = csp.tile([128, KT, TOKCHUNK], BF16, tag="cs", name=f"cs{c}")
                # cs is on the critical path of iteration 0 (k/v for all j
                # tiles); alternate queues so no single queue serializes it
                (nc.scalar if c == 2 else nc.sync).dma_start(cs[:], ct_e[c])
                state["cs"] = cs

            def dma_xs():
                if c == 0:
                    state["xs"] = xs0
                    return
                xs = xsp.tile([128, KT, TOKCHUNK], BF16, tag="xs", name=f"xs{c}")
                (nc.sync if c == 2 else nc.scalar).dma_start(xs[:], xt_e[c])
                state["xs"] = xs

            def proj(kind, half, sub, nsub=4):
                """Sub-piece of one q/k projection half: 2 accumulating
                N=512 matmuls; evacuate on the last sub."""
                hsl = slice(128 * half, 128 * (half + 1))
                w, src, dst = {
                    "q": (wq_sb, "xs", qT_sb),
                    "k": (wk_sb, "cs", kT_sb),
                }[kind]
                key = ("p", kind, half)
                if sub == 0:
                    state[key] = ps_m.tile([128, TOKCHUNK], F32, tag="pm",
                                           name=f"p{kind}{c}_{half}")
                p = state[key]
                tpu = KT // nsub
                for t in range(tpu * sub, tpu * (sub + 1)):
                    nc.tensor.matmul(p[:], w[:, t, hsl], state[src][:, t, :],
                                     start=(t == 0), stop=(t == KT - 1))
                if sub == nsub - 1:
                    gsl = slice(c * TOKCHUNK, (c + 1) * TOKCHUNK)
                    nc.vector.tensor_copy(dst[:, half, gsl], p[:])

            def vproj(jj, sub, nsub=2):
                """Sub-piece of the direct-transposed V projection for one
                128-token block: lhsT = ctxT block, rhs = Wv, so the output
                lands as [token, feature] with no transpose anywhere."""
                jt = c * 4 + jj
                key = ("v", jj)
                if sub == 0:
                    state[key] = ps_m.tile([128, TOKCHUNK], F32, tag="pm",
                                           name=f"pv{c}_{jj}")
                p = state[key]
                tpu = KT // nsub
                bsl = slice(128 * jj, 128 * (jj + 1))
                for t in range(tpu * sub, tpu * (sub + 1)):
                    nc.tensor.matmul(p[0:128, 0:HDC],
                                     state["cs"][:, t, bsl], wv_sb[:, t, :],
                                     start=(t == 0), stop=(t == KT - 1))
                if sub == nsub - 1:
                    dst = (v_all[:].rearrange("p h (j c) -> p h j c", c=65)
                           [:, :, jt, 0:64])
                    nc.vector.tensor_copy(
                        dst, p[:, 0:HDC].rearrange("p (h d) -> p h d", d=64))

            # (deadline_group, closure): deadline = global group index by
            # which the piece must be EMITTED (program order = dependencies).
            # chunk c: k half hh feeds iteration (i=0, hp=hh) groups 16hh+4c;
            # v block jj feeds group 4c+jj of iteration 0; q half hh feeds
            # iteration (i=c, hp=hh) = group 16*(2c+hh).  Sub-piece deadlines
            # are SPREAD across the preceding groups so no single group ever
            # pops a whole unit as a burst (which would stall the exp
            # stream); DMAs are issued as early as possible (the queues
            # stream in the background, destination buffers are dedicated).
            # all subs of one unit share a deadline so they stay ADJACENT in
            # the queue (front-pop discipline then guarantees nothing can
            # allocate misc-PSUM between a unit's held sub-pieces); the
            # lookahead boost in the group loop drains them one per group
            # ahead of the deadline instead of bursting at it
            # cs DMAs issue first (groups 0-2); xs DMAs are deferred so
            # they queue BEHIND all cs chunks on both queues (q-units only
            # read xs much later)
            pieces = [(max(0, c - 1), dma_cs),
                      (0 if c == 0 else 4 + c, dma_xs)]
            for half in range(2):
                kdl = 4 * c if half == 0 else 20 + 16 * c
                for sub in range(4):
                    pieces.append((kdl,
                                   lambda hh=half, s=sub: proj("k", hh, s)))
            for jj in range(4):
                # attn@V pops DEPTH groups late, so block jj is not read
                # before group 4c+jj+1; the +1 spreads iter-0's v work
                for sub in range(2):
                    pieces.append((min(4 * c + jj + 1, 15),
                                   lambda j=jj, s=sub: vproj(j, s)))
            for half in range(2):
                qdl = 16 * c if half == 0 else 56 + 16 * c
                for sub in range(4):
                    pieces.append((qdl,
                                   lambda hh=half, s=sub: proj("q", hh, s)))
            return pieces

        # ---------- out-projection pieces for one finished i-chunk
        # (contract K=256 over both head-pairs' on-tiles; hp1 None for the
        # final i-chunk, whose hp=1 part ships via the tail)
        def outproj_pieces(i, on0, on1, last):
            isl = slice(i * ICHUNK, (i + 1) * ICHUNK)

            def one(d8):
                po = ps_m.tile([128, ICHUNK], F32, tag="pm",
                               name=f"po{i}_{d8}")
                dsl = slice(d8 * 128, (d8 + 1) * 128)
                if on1 is not None:
                    nc.tensor.matmul(po[:], wo_sb[:, 0, dsl], on0[:],
                                     start=True, stop=False)
                    nc.tensor.matmul(po[:], wo_sb[:, 1, dsl], on1[:],
                                     start=False, stop=True)
                else:
                    nc.tensor.matmul(po[:], wo_sb[:, 0, dsl], on0[:],
                                     start=True, stop=True)
                ob = obp.tile([128, ICHUNK], F16, tag="ob", name=f"ob{i}_{d8}",
                               bufs=8)
                nc.vector.tensor_copy(ob[:], po[:])
                # bulk output rides the SWDGE queue so the latency-critical
                # small normalize DMAs on the SP queue never sit behind
                # 128KB writes; only i==2 (popped in iteration 7, after the
                # last normalize chain has issued) goes on SP so its writes
                # don't drain past the new tail on the slower SWDGE
                eng = nc.sync if (last or i == 2) else nc.gpsimd
                eng.dma_start(out_v[:, d8, isl], ob[:])

            return [lambda d=d8: one(d) for d8 in range(8)]

        # ---------- emission schedule ----------
        # One strictly-ordered queue of (deadline, seq, closure) for ALL
        # misc-PSUM work; stable deadline order means a multi-sub unit's
        # pieces stay adjacent, so at most one foreign piece (inserted at a
        # later deadline) sits between consecutive allocations -> the 2-buf
        # misc pool can never clobber a held accumulation.
        cq = []
        seq = [0]

        def push(dl, fn):
            insort(cq, (dl, seq[0], fn))
            seq[0] += 1

        for c in range(NCH):
            for dl, fn in chunk_pieces(c):
                push(dl, fn)

        from collections import deque
        # hp-major iteration order: all hp=0 i-chunks first, then hp=1.
        # k-half1 / q-half1 construction then spreads across the
        # underloaded middle iterations instead of clustering at every
        # hp boundary (the hp=0 iterations only need half the k/q work).
        iters = [(i, hp) for hp in range(2) for i in range(N // ICHUNK)]
        n_deadline0 = sum(1 for dl, _, _ in cq if dl == 0)
        # attn@V consumption runs DEPTH groups behind its scores/exp so the
        # PE never sits at an attn@V whose exp hasn't finished (the ex pool's
        # 4 buffers make depth 3 safe: ex[g] is overwritten by scores g+4,
        # and attn@V g is consumed at g+3).  Depth 3 also gives the acc-
        # recycle copies at iteration boundaries a full group of slack.
        DEPTH = 4
        pend = deque()        # emitted-but-unconsumed attn@V closures
        cq_rate = [(len(cq) - n_deadline0) / 112.0]
        cq_credit = [0.0]
        on_tiles = {}

        for it, (i, hp) in enumerate(iters):
            isl = slice(i * ICHUNK, (i + 1) * ICHUNK)
            acc = ps_a.tile([128, 2 * ICHUNK], F32, tag="acc",
                            name=f"acc{i}_{hp}")

            def scores(hp, i, j, isl):
                ss = ps_s.tile([128, 2 * 512], F32, tag="ss",
                               name=f"ss{i}_{hp}_{j}")
                jsl = slice(j * 128, (j + 1) * 128)
                for h in range(2):
                    hs = slice(64 * h, 64 * h + 64)
                    nc.tensor.matmul(ss[:, 512 * h: 512 * (h + 1)],
                                     kT_sb[hs, hp, jsl], qT_sb[hs, hp, isl],
                                     start=True, stop=True)
                ex = exp.tile([128, 2 * 512], BF16, tag="ex",
                              name=f"ex{i}_{hp}_{j}")
                nc.scalar.activation(ex[:], ss[:], AF.Exp)
                return ex

            def attnv(hp, j, ex, acc):
                for h in range(2):
                    nc.tensor.matmul(
                        acc[0:65, ICHUNK * h: ICHUNK * (h + 1)],
                        v_all[:, 2 * hp + h, 65 * j: 65 * j + 65],
                        ex[:, 512 * h: 512 * (h + 1)],
                        start=(j == 0), stop=(j == NJT - 1))

            for g in range(NJT):
                gg = it * NJT + g
                # correctness: pieces this group's scores/attnv read must be
                # emitted first (program order defines dependencies)
                while cq and cq[0][0] <= gg:
                    cq.pop(0)[2]()
                # consume the attn@V from DEPTH groups ago (its normalize
                # closure, if any, is composed onto it) BEFORE this group's
                # scores: the scores matmul waits on the exp(g-2) bank
                # recycle, and parking the PE on that semaphore with no
                # work in between resets the p-state ramp every group
                if len(pend) >= DEPTH:
                    pend.popleft()()
                ex = scores(hp, i, g, isl)
                pend.append(lambda hh=hp, gg_=g, e=ex, a=acc:
                            attnv(hh, gg_, e, a))
                if it == 4 and g == DEPTH:
                    # last i-chunk's hp=0 contribution: its on-tile became
                    # ready at this iteration's g=0 flush (hp-major order),
                    # and iterations 4-5 have spare filler capacity while
                    # iteration 7 is loaded with outproj(2) already.  +3
                    # keeps the first piece clear of the normalize's DRAM
                    # round-trip (~3us) so the PE never parks on it.
                    for k, fn in enumerate(outproj_pieces(3, on_tiles[(3, 0)],
                                                          None, True)):
                        push(gg + 3 + 2 * k, fn)
                # paced fillers: ~1 micro-piece per group; when a deadline
                # looms within 8 groups, guarantee at least one pop so a
                # unit's subs drain gradually instead of bursting at the
                # deadline (which would stall the exp stream)
                cq_credit[0] += cq_rate[0]
                if cq and cq[0][0] <= gg + 8:
                    cq_credit[0] = max(cq_credit[0], 1.0)
                while cq and cq_credit[0] >= 1.0:
                    cq_credit[0] -= 1.0
                    cq.pop(0)[2]()
            if it == len(iters) - 1:
                while pend:
                    pend.popleft()()
                # ---- tail: ship the raw hp=1 accumulator (unnormalized
                # per-head products + rowsum row) for the last i-chunk;
                # host divides and applies the small out-projection.  Two
                # casts (DVE + the now-idle ScalarE) + two small DMAs.
                tlA = nrm.tile([128, ICHUNK], F16, tag="tlA", name="tlA")
                tlB = nrm.tile([128, ICHUNK], F16, tag="tlB", name="tlB")
                nc.vector.tensor_copy(tlA[0:65, :], acc[0:65, 0:ICHUNK])
                nc.scalar.copy(tlB[0:65, :], acc[0:65, ICHUNK:])
                nc.sync.dma_start(tl_e[0], tlA[0:65, :])
                nc.scalar.dma_start(tl_e[1], tlB[0:65, :])
                continue

            # ---- normalization: deferred into the next iteration's group-0
            # slot so this iteration's last attn@V flows straight into the
            # next one's scores with filler in between ----
            def make_norm(acc=acc, i=i, hp=hp, it=it):
                def norm():
                    accs = nrm.tile([128, 2 * ICHUNK], F32R, tag="accs",
                                    name=f"accs{i}_{hp}")
                    with nc.allow_low_precision(
                            reason="attn out + softmax denom fp32r"):
                        nc.vector.tensor_copy(accs[0:65, 0:ICHUNK],
                                              acc[0:65, 0:ICHUNK])
                        nc.vector.tensor_copy(accs[0:65, ICHUNK:],
                                              acc[0:65, ICHUNK:])
                    # rowsum [1, 1024]@p64 -> DRAM -> [128, 8] -> recip ->
                    # DRAM -> broadcast-DMA to 128 partitions (no PE/PSUM).
                    # The relayout bounce is essential: DVE RECIPROCAL is
                    # ~6.4ns/elem PER PARTITION (6.5us on [1, 1024] vs 203ns
                    # on [128, 8]).
                    rs_d = drp.tile([2 * ICHUNK], F32R, tag="rs_d",
                                    name=f"rs_d{i}_{hp}")
                    nc.sync.dma_start(rs_d[:], accs[64:65, :])
                    rs128 = nrm.tile([128, 8], F32R, tag="rs128",
                                     name=f"rs128{i}_{hp}")
                    nc.sync.dma_start(
                        rs128[:], rs_d[:].rearrange("(p a) -> p a", p=128))
                    rr128 = nrm.tile([128, 8], F32R, tag="rr128",
                                     name=f"rr128{i}_{hp}")
                    with nc.allow_low_precision(
                            reason="softmax denom recip fp32r"):
                        nc.vector.reciprocal(rr128[:], rs128[:])
                    rr_d = drp.tile([2 * ICHUNK], F32R, tag="rr_d",
                                    name=f"rr_d{i}_{hp}")
                    nc.sync.dma_start(
                        rr_d[:].rearrange("(p a) -> p a", p=128), rr128[:])
                    bcs = nrm.tile([128, 2 * ICHUNK], F32R, tag="bcs",
                                   name=f"bcs{i}_{hp}")
                    nc.sync.dma_start(
                        bcs[:],
                        rr_d[:].unsqueeze(0).broadcast_to([128, 2 * ICHUNK]))
                    on = nrm.tile([128, ICHUNK], BF16, tag="on",
                                  name=f"on{i}_{hp}", bufs=6)
                    with nc.allow_low_precision(
                            reason="attn out normalize bf16"):
                        for h in range(2):
                            nc.vector.tensor_mul(
                                on[64 * h: 64 * h + 64, :],
                                accs[0:64, ICHUNK * h: ICHUNK * (h + 1)],
                                bcs[0:64, ICHUNK * h: ICHUNK * (h + 1)])
                    on_tiles[(i, hp)] = on
                    if hp == 1:
                        # queue i's out-projection into upcoming filler
                        # slots (8 pieces, due progressively).  First piece
                        # +4 groups after the boundary so the normalize's
                        # DRAM round-trip (~3us) completes before the PE
                        # reaches the first outproj matmul in program order.
                        for k, fn in enumerate(
                                outproj_pieces(i, on_tiles[(i, 0)], on,
                                               False)):
                            push((it + 1) * NJT + 4 + 2 * k, fn)
                return norm
            # ride the normalize right behind this iteration's last attn@V
            last_attnv = pend.pop()
            pend.append(lambda f=last_attnv, n=make_norm(): (f(), n()))

        while cq:
            cq.pop(0)[2]()

    nc.compile()
    return nc


def _get_program():
    global _PROGRAM
    if _PROGRAM is None:
        _PROGRAM = _build_program()
    return _PROGRAM


def _prepare_in_maps(x, context, Wq, Wk, Wv, Wo, bo):
    import ml_dtypes
    bf16 = ml_dtypes.bfloat16

    x = np.asarray(x, dtype=np.float32)
    context = np.asarray(context, dtype=np.float32)
    Wq = np.asarray(Wq, dtype=np.float32)
    Wk = np.asarray(Wk, dtype=np.float32)
    Wv = np.asarray(Wv, dtype=np.float32)
    Wo = np.asarray(Wo, dtype=np.float32)
    Wk_s = Wk * np.float32(SCALE)

    def featmaj(a):  # [N, D] -> [NCH, 128, KT, TOKCHUNK] bf16
        t = a.T.reshape(KT, 128, NCH, TOKCHUNK).transpose(2, 1, 0, 3)
        return np.ascontiguousarray(t).astype(bf16)

    xT = [featmaj(x[b]) for b in range(B)]
    cT = [featmaj(context[b]) for b in range(B)]

    in_maps = []
    for c in range(NCORES):
        b, g = c // 4, c % 4
        cs = slice(g * HDC, (g + 1) * HDC)
        in_maps.append({
            "xt": xT[b],
            "ct": cT[b],
            "wq": np.ascontiguousarray(Wq[:, cs]).astype(bf16),
            "wk": np.ascontiguousarray(Wk_s[:, cs]).astype(bf16),
            "wv": np.ascontiguousarray(Wv[:, cs]).astype(bf16),
            "wo": np.ascontiguousarray(Wo[cs, :]).astype(bf16),
        })
    return in_maps


def _gather(results, bo, Wo):
    bo = np.asarray(bo, dtype=np.float32)
    Wo = np.asarray(Wo, dtype=np.float32)
    last = slice(NT - ICHUNK, NT)
    outs = []
    for b in range(B):
        acc = results[4 * b]["out"].astype(np.float64)
        for g in range(1, 4):
            acc += results[4 * b + g]["out"]
        for g in range(4):
            r = results[4 * b + g]
            tl = r["tl"].astype(np.float64)          # [2, 65, ICHUNK]
            for h in range(2):
                attn = tl[h, 0:64] / tl[h, 64]       # [64, ICHUNK]
                row0 = g * HDC + 128 + 64 * h
                wo_h = Wo[row0: row0 + 64]           # [64, D]
                acc[:, last] += wo_h.T @ attn
        outs.append(acc.T.astype(np.float32) + bo)
    return np.stack(outs).reshape(B, N, D)


def kernel(x, context, Wq, Wk, Wv, Wo, bo):
    from concourse.bass_utils import run_bass_kernel_spmd

    in_maps = _prepare_in_maps(x, context, Wq, Wk, Wv, Wo, bo)
    nc = _get_program()
    res = run_bass_kernel_spmd(nc, in_maps, list(range(NCORES)))
    return _gather(res.results, bo, Wo)



# revision 37
# speedup vs baseline: 1.0202x; 1.0122x over previous
"""Cross-attention kernel for 8 Trainium2 NeuronCores.

Sharding: (batch x head-group) -- core c handles batch c//4 and heads
4*(c%4)..4*(c%4)+3 (two head-pairs of 128 features each).  Each core reads
only its batch's x/context (8MB instead of 16MB) and writes a [1024, 2048]
fp16 partial; the host sums the 4 partials per batch and adds the bias.
Compute per core is identical to a pure-Megatron head split.

Dataflow is feature-major ("transposed") end to end:
  xT/ctxT [1024, 2048] -> qT/kT [256(hd), 2048] -> scoresT [j, i]
V is produced directly in [token, feature] layout by swapping the matmul
operands (lhsT = a 128-token block of ctxT, rhs = Wv), so nothing is ever
transposed on any engine.  The softmax denominator falls out of the attn@V
matmul as a 65th output row (ones column appended to V).  Matmul operands
are bf16 (fp32 PSUM accumulate).

Iterations run i-outer / head-pair-inner so the out-projection can contract
over both head-pairs (K=256) into one PSUM accumulation.

The emission order is a manual software pipeline tuned to hide the PE stream
inside the ScalarE exp shadow (1147ns per group vs ~640ns of scores+attn@V):
score matmuls for group g+1 are emitted before attn@V of group g
(double-buffered score PSUM), and all projection / out-projection work is
chopped into ~2-matmul micro-pieces that are deadline-scheduled into the
group loop as PE filler, one piece per group.  All misc-PSUM users live in
one strictly-ordered queue sharing a double-buffered bank pair; multi-piece
projection units hold their PSUM tile across their sub-pieces (at most one
foreign allocation can interleave, so two buffers suffice).  The prologue
(weights + chunk 0) is split across the two HWDGE queues (SP + Activation);
steady-state input chunks ride the Activation queue (fresh destination tiles
never stall the exp stream), latency-sensitive small DMAs ride SP.  The
softmax reciprocal is re-laid out to 128 partitions via a DRAM bounce.
"""

import numpy as np

B, N, D, H, DH = 2, 2048, 1024, 16, 64
SCALE = DH ** -0.5
NT = N                  # tokens per core (one batch)
HDC = 4 * DH            # 256 head-dims per core (4 heads = 2 head-pairs)
NCORES = 8

TOKCHUNK = 512          # projection chunk (4 chunks)
ICHUNK = 512            # query chunk in attention (4 per core)
NJT = N // 128          # 16 j-tiles
KT = D // 128           # 8 contraction tiles for projections
NCH = NT // TOKCHUNK    # 4

_PROGRAM = None


def _build_program():
    from contextlib import ExitStack
    from bisect import insort
    import concourse.mybir as mybir
    import concourse.tile as tile
    from concourse import bacc

    F32 = mybir.dt.float32
    F32R = mybir.dt.float32r
    F16 = mybir.dt.float16
    BF16 = mybir.dt.bfloat16
    AF = mybir.ActivationFunctionType

    nc = bacc.Bacc(None, target_bir_lowering=False)

    xt_e = nc.declare_dram_parameter("xt", [NCH, 128, KT, TOKCHUNK], BF16,
                                     isOutput=False)
    ct_e = nc.declare_dram_parameter("ct", [NCH, 128, KT, TOKCHUNK], BF16,
                                     isOutput=False)
    wq_e = nc.declare_dram_parameter("wq", [D, HDC], BF16, isOutput=False)
    wk_e = nc.declare_dram_parameter("wk", [D, HDC], BF16, isOutput=False)
    wv_e = nc.declare_dram_parameter("wv", [D, HDC], BF16, isOutput=False)
    wo_e = nc.declare_dram_parameter("wo", [HDC, D], BF16, isOutput=False)
    out_e = nc.declare_dram_parameter("out", [D, NT], F16, isOutput=True)
    # last i-chunk's hp=1 contribution ships as the raw attn accumulator
    # (64 v-dims + rowsum row, per head); the host divides and applies the
    # tiny out-projection (kills the 16-matmul + 2MB-DMA tail drain)
    tl_e = nc.declare_dram_parameter("tl", [2, 65, ICHUNK], F16,
                                     isOutput=True)

    wq_v = wq_e[:].rearrange("(t p) m -> p t m", p=128)     # [128, 8, 256]
    wk_v = wk_e[:].rearrange("(t p) m -> p t m", p=128)
    wv_v = wv_e[:].rearrange("(t p) m -> p t m", p=128)
    wo_v = wo_e[:].rearrange("(t p) m -> p t m", p=128)     # [128, 2, 1024]
    out_v = out_e[:].rearrange("(t p) n -> p t n", p=128)   # [128, 8, 2048]

    with tile.TileContext(nc) as tc, ExitStack() as ctx:
        const = ctx.enter_context(tc.tile_pool(name="const", bufs=1))
        wpool = ctx.enter_context(tc.tile_pool(name="wpool", bufs=1))
        xsp = ctx.enter_context(tc.tile_pool(name="xsp", bufs=NCH))
        csp = ctx.enter_context(tc.tile_pool(name="csp", bufs=NCH))
        qkp = ctx.enter_context(tc.tile_pool(name="qkp", bufs=1))
        vsb = ctx.enter_context(tc.tile_pool(name="vsb", bufs=1))
        exp = ctx.enter_context(tc.tile_pool(name="exp", bufs=5))
        nrm = ctx.enter_context(tc.tile_pool(name="nrm", bufs=2))
        obp = ctx.enter_context(tc.tile_pool(name="obp", bufs=4))
        drp = ctx.enter_context(tc.tile_pool(name="drp", bufs=2, space="DRAM"))
        ps_s = ctx.enter_context(tc.tile_pool(name="ps_s", bufs=2, space="PSUM"))
        ps_a = ctx.enter_context(tc.tile_pool(name="ps_a", bufs=1, space="PSUM"))
        ps_m = ctx.enter_context(tc.tile_pool(name="ps_m", bufs=2, space="PSUM"))

        # --- constants ---
        ones32 = const.tile([128, 128], F32, tag="ones32", name="ones32")
        nc.gpsimd.memset(ones32[:], 1.0)
        oneb = const.tile([128, 128], BF16, tag="oneb", name="oneb")
        nc.gpsimd.memset(oneb[:], 1.0)
        zerob = const.tile([128, 128], BF16, tag="zerob", name="zerob")
        nc.gpsimd.memset(zerob[:], 0.0)
        wsc = const.tile([128, 128], F32, tag="wsc", name="wsc")

        # --- weights + chunk-0 input, split across the two HWDGE queues so
        # the prologue DMA runs ~2x faster.  Critical order: k-projection
        # (wk + cs0) unblocks first, then v (wv), then q (wq + xs0). ---
        wq_sb = wpool.tile([128, KT, HDC], BF16, tag="wq_sb", name="wq_sb")
        wk_sb = wpool.tile([128, KT, HDC], BF16, tag="wk_sb", name="wk_sb")
        wv_sb = wpool.tile([128, KT, HDC], BF16, tag="wv_sb", name="wv_sb")
        wo_sb = wpool.tile([128, 2, D], BF16, tag="wo_sb", name="wo_sb")
        xs0 = xsp.tile([128, KT, TOKCHUNK], BF16, tag="xs", name="xs0")
        cs0 = csp.tile([128, KT, TOKCHUNK], BF16, tag="cs", name="cs0")
        # Prologue DMA layout is latency-ordered for the first k/q
        # projection HALVES (feature half 0): weights split by FEATURE half
        # (each projection half then depends on a single queue's piece, not
        # both), cs0/xs0 split into t-quarters matching proj sub-pieces so
        # sub 0 can start as soon as t0-1 land.  Only group-0-critical data
        # rides the prologue: the hp=1 weight halves (wk/wq f1, first used
        # at group 20+) and wo (group ~84) are deferred into the piece
        # queue.  cs1 rides sync right behind wq f0 (k-chunk-1 feeds
        # groups 4-7, which arrive early while iteration 0 is PE-bound).
        HH = HDC // 2
        nc.sync.dma_start(wk_sb[:, :, 0:HH], wk_v[:, :, 0:HH])
        nc.scalar.dma_start(wk_sb[:, :, HH:], wk_v[:, :, HH:])
        for qq in range(4):
            eng = nc.sync if qq < 2 else nc.scalar
            eng.dma_start(cs0[:, 2 * qq: 2 * qq + 2],
                          ct_e[0, :, 2 * qq: 2 * qq + 2])
        nc.sync.dma_start(wq_sb[:, :, 0:HH], wq_v[:, :, 0:HH])
        nc.scalar.dma_start(wq_sb[:, :, HH:], wq_v[:, :, HH:])
        for qq in range(4):
            eng = nc.sync if qq < 2 else nc.scalar
            eng.dma_start(xs0[:, 2 * qq: 2 * qq + 2],
                          xt_e[0, :, 2 * qq: 2 * qq + 2])
        nc.sync.dma_start(wv_sb[:, :, 0:HH], wv_v[:, :, 0:HH])
        nc.scalar.dma_start(wv_sb[:, :, HH:], wv_v[:, :, HH:])
        nc.scalar.dma_start(wo_sb[:], wo_v)

        # --- PE warm-up: dummy matmuls while the prologue DMAs stream, so
        # the HAM clock-gate ramps before the first real projection matmuls
        # (which sit on the first-exp critical path).  The two HWDGE queues
        # share the 16 DMA engines (~300GB/s aggregate), so the ~2.5MB of
        # group-0-critical data lands at ~15-16us; the warm-up must span
        # until then or the idle gap resets the PE p-state and the whole
        # prologue projection chain runs at 0.65-1.2GHz. ---
        pw = ps_m.tile([128, 128], F32, tag="pm", name="warm_ps")
        # moving operand is all-zero: the ramp only needs the PE busy,
        # and zero products minimize array toggling so the warm-up burst
        # doesn't trip the HAM power throttle right as real work starts
        for r in range(48):
            nc.tensor.matmul(pw[:], oneb[:], zerob[:],
                             start=(r == 0), stop=(r == 47))
        nc.vector.tensor_copy(wsc[:], pw[:])

        # --- persistent activations ---
        qT_sb = qkp.tile([128, 2, NT], BF16, tag="qT_sb", name="qT_sb")
        kT_sb = qkp.tile([128, 2, NT], BF16, tag="kT_sb", name="kT_sb")
        # V^T for all 4 heads in one tile: [token, head, jtile*(64 dims+one)]
        v_all = vsb.tile([128, 4, NJT * 65], BF16, tag="v_all", name="v_all")
        ones_v = v_all[:].rearrange("p h (j c) -> p (h j) c", c=65)[:, :, 64]
        nc.vector.tensor_copy(ones_v, ones32[:, 0:4 * NJT])

        # ---------- projection chunk emission, chopped into ~2-matmul
        # micro-pieces that fit the per-group exp-shadow filler budget.
        # Multi-sub units hold their misc-PSUM tile across sub-pieces.
        def chunk_pieces(c):
            state = {}

            def dma_cs():
                if c == 0:
                    state["cs"] = cs0
                    return
                cs = csp.tile([128, KT, TOKCHUNK], BF16, tag="cs", name=f"cs{c}")
                # cs is on the critical path of iteration 0 (k/v for all j
                # tiles); alternate queues so no single queue serializes it
                (nc.scalar if c == 2 else nc.sync).dma_start(cs[:], ct_e[c])
                state["cs"] = cs

            def dma_xs():
                if c == 0:
                    state["xs"] = xs0
                    return
                xs = xsp.tile([128, KT, TOKCHUNK], BF16, tag="xs", name=f"xs{c}")
                (nc.sync if c == 2 else nc.scalar).dma_start(xs[:], xt_e[c])
                state["xs"] = xs

            def proj(kind, half, sub, nsub=4):
                """Sub-piece of one q/k projection half: 2 accumulating
                N=512 matmuls; evacuate on the last sub."""
                hsl = slice(128 * half, 128 * (half + 1))
                w, src, dst = {
                    "q": (wq_sb, "xs", qT_sb),
                    "k": (wk_sb, "cs", kT_sb),
                }[kind]
                key = ("p", kind, half)
                if sub == 0:
                    state[key] = ps_m.tile([128, TOKCHUNK], F32, tag="pm",
                                           name=f"p{kind}{c}_{half}")
                p = state[key]
                tpu = KT // nsub
                for t in range(tpu * sub, tpu * (sub + 1)):
                    nc.tensor.matmul(p[:], w[:, t, hsl], state[src][:, t, :],
                                     start=(t == 0), stop=(t == KT - 1))
                if sub == nsub - 1:
                    gsl = slice(c * TOKCHUNK, (c + 1) * TOKCHUNK)
                    nc.vector.tensor_copy(dst[:, half, gsl], p[:])

            def vproj(jj, sub, nsub=2):
                """Sub-piece of the direct-transposed V projection for one
                128-token block: lhsT = ctxT block, rhs = Wv, so the output
                lands as [token, feature] with no transpose anywhere."""
                jt = c * 4 + jj
                key = ("v", jj)
                if sub == 0:
                    state[key] = ps_m.tile([128, TOKCHUNK], F32, tag="pm",
                                           name=f"pv{c}_{jj}")
                p = state[key]
                tpu = KT // nsub
                bsl = slice(128 * jj, 128 * (jj + 1))
                for t in range(tpu * sub, tpu * (sub + 1)):
                    nc.tensor.matmul(p[0:128, 0:HDC],
                                     state["cs"][:, t, bsl], wv_sb[:, t, :],
                                     start=(t == 0), stop=(t == KT - 1))
                if sub == nsub - 1:
                    dst = (v_all[:].rearrange("p h (j c) -> p h j c", c=65)
                           [:, :, jt, 0:64])
                    nc.vector.tensor_copy(
                        dst, p[:, 0:HDC].rearrange("p (h d) -> p h d", d=64))

            # (deadline_group, closure): deadline = global group index by
            # which the piece must be EMITTED (program order = dependencies).
            # chunk c: k half hh feeds iteration (i=0, hp=hh) groups 16hh+4c;
            # v block jj feeds group 4c+jj of iteration 0; q half hh feeds
            # iteration (i=c, hp=hh) = group 16*(2c+hh).  Sub-piece deadlines
            # are SPREAD across the preceding groups so no single group ever
            # pops a whole unit as a burst (which would stall the exp
            # stream); DMAs are issued as early as possible (the queues
            # stream in the background, destination buffers are dedicated).
            # all subs of one unit share a deadline so they stay ADJACENT in
            # the queue (front-pop discipline then guarantees nothing can
            # allocate misc-PSUM between a unit's held sub-pieces); the
            # lookahead boost in the group loop drains them one per group
            # ahead of the deadline instead of bursting at it
            # cs DMAs issue first (groups 0-2); xs DMAs are deferred so
            # they queue BEHIND all cs chunks on both queues (q-units only
            # read xs much later)
            pieces = [(max(0, c - 1), dma_cs),
                      (0 if c == 0 else 4 + c, dma_xs)]
            for half in range(2):
                kdl = 4 * c if half == 0 else 20 + 16 * c
                for sub in range(4):
                    pieces.append((kdl,
                                   lambda hh=half, s=sub: proj("k", hh, s)))
            for jj in range(4):
                # attn@V pops DEPTH groups late, so block jj is not read
                # before group 4c+jj+1; the +1 spreads iter-0's v work
                for sub in range(2):
                    pieces.append((min(4 * c + jj + 1, 15),
                                   lambda j=jj, s=sub: vproj(j, s)))
            for half in range(2):
                qdl = 16 * c if half == 0 else 56 + 16 * c
                for sub in range(4):
                    pieces.append((qdl,
                                   lambda hh=half, s=sub: proj("q", hh, s)))
            return pieces

        # ---------- out-projection pieces for one finished i-chunk
        # (contract K=256 over both head-pairs' on-tiles; hp1 None for the
        # final i-chunk, whose hp=1 part ships via the tail)
        def outproj_pieces(i, on0, on1, last):
            isl = slice(i * ICHUNK, (i + 1) * ICHUNK)

            def one(d8):
                po = ps_m.tile([128, ICHUNK], F32, tag="pm",
                               name=f"po{i}_{d8}")
                dsl = slice(d8 * 128, (d8 + 1) * 128)
                if on1 is not None:
                    nc.tensor.matmul(po[:], wo_sb[:, 0, dsl], on0[:],
                                     start=True, stop=False)
                    nc.tensor.matmul(po[:], wo_sb[:, 1, dsl], on1[:],
                                     start=False, stop=True)
                else:
                    nc.tensor.matmul(po[:], wo_sb[:, 0, dsl], on0[:],
                                     start=True, stop=True)
                ob = obp.tile([128, ICHUNK], F16, tag="ob", name=f"ob{i}_{d8}",
                               bufs=6)
                nc.vector.tensor_copy(ob[:], po[:])
                # bulk output rides the SWDGE queue so the latency-critical
                # small normalize DMAs on the SP queue never sit behind
                # 128KB writes; only i==2 (popped in iteration 7, after the
                # last normalize chain has issued) goes on SP so its writes
                # don't drain past the new tail on the slower SWDGE
                eng = nc.sync if (last or i == 2) else nc.gpsimd
                eng.dma_start(out_v[:, d8, isl], ob[:])

            return [lambda d=d8: one(d) for d8 in range(8)]

        # ---------- emission schedule ----------
        # One strictly-ordered queue of (deadline, seq, closure) for ALL
        # misc-PSUM work; stable deadline order means a multi-sub unit's
        # pieces stay adjacent, so at most one foreign piece (inserted at a
        # later deadline) sits between consecutive allocations -> the 2-buf
        # misc pool can never clobber a held accumulation.
        cq = []
        seq = [0]

        def push(dl, fn):
            insort(cq, (dl, seq[0], fn))
            seq[0] += 1

        for c in range(NCH):
            for dl, fn in chunk_pieces(c):
                push(dl, fn)

        from collections import deque
        # hp-major iteration order: all hp=0 i-chunks first, then hp=1.
        # k-half1 / q-half1 construction then spreads across the
        # underloaded middle iterations instead of clustering at every
        # hp boundary (the hp=0 iterations only need half the k/q work).
        iters = [(i, hp) for hp in range(2) for i in range(N // ICHUNK)]
        n_deadline0 = sum(1 for dl, _, _ in cq if dl == 0)
        # attn@V consumption runs DEPTH groups behind its scores/exp so the
        # PE never sits at an attn@V whose exp hasn't finished (the ex pool's
        # 4 buffers make depth 3 safe: ex[g] is overwritten by scores g+4,
        # and attn@V g is consumed at g+3).  Depth 3 also gives the acc-
        # recycle copies at iteration boundaries a full group of slack.
        DEPTH = 4
        pend = deque()        # emitted-but-unconsumed attn@V closures
        cq_rate = [(len(cq) - n_deadline0) / 112.0]
        cq_credit = [0.0]
        on_tiles = {}

        for it, (i, hp) in enumerate(iters):
            isl = slice(i * ICHUNK, (i + 1) * ICHUNK)
            acc = ps_a.tile([128, 2 * ICHUNK], F32, tag="acc",
                            name=f"acc{i}_{hp}")

            def scores(hp, i, j, isl):
                ss = ps_s.tile([128, 2 * 512], F32, tag="ss",
                               name=f"ss{i}_{hp}_{j}")
                jsl = slice(j * 128, (j + 1) * 128)
                for h in range(2):
                    hs = slice(64 * h, 64 * h + 64)
                    nc.tensor.matmul(ss[:, 512 * h: 512 * (h + 1)],
                                     kT_sb[hs, hp, jsl], qT_sb[hs, hp, isl],
                                     start=True, stop=True)
                ex = exp.tile([128, 2 * 512], BF16, tag="ex",
                              name=f"ex{i}_{hp}_{j}")
                nc.scalar.activation(ex[:], ss[:], AF.Exp)
                return ex

            def attnv(hp, j, ex, acc):
                for h in range(2):
                    nc.tensor.matmul(
                        acc[0:65, ICHUNK * h: ICHUNK * (h + 1)],
                        v_all[:, 2 * hp + h, 65 * j: 65 * j + 65],
                        ex[:, 512 * h: 512 * (h + 1)],
                        start=(j == 0), stop=(j == NJT - 1))

            for g in range(NJT):
                gg = it * NJT + g
                # correctness: pieces this group's scores/attnv read must be
                # emitted first (program order defines dependencies)
                while cq and cq[0][0] <= gg:
                    cq.pop(0)[2]()
                # consume the attn@V from DEPTH groups ago (its normalize
                # closure, if any, is composed onto it) BEFORE this group's
                # scores: the scores matmul waits on the exp(g-2) bank
                # recycle, and parking the PE on that semaphore with no
                # work in between resets the p-state ramp every group
                if len(pend) >= DEPTH:
                    pend.popleft()()
                ex = scores(hp, i, g, isl)
                pend.append(lambda hh=hp, gg_=g, e=ex, a=acc:
                            attnv(hh, gg_, e, a))
                if it == 4 and g == DEPTH:
                    # last i-chunk's hp=0 contribution: its on-tile became
                    # ready at this iteration's g=0 flush (hp-major order),
                    # and iterations 4-5 have spare filler capacity while
                    # iteration 7 is loaded with outproj(2) already.  +3
                    # keeps the first piece clear of the normalize's DRAM
                    # round-trip (~3us) so the PE never parks on it.
                    for k, fn in enumerate(outproj_pieces(3, on_tiles[(3, 0)],
                                                          None, True)):
                        push(gg + 3 + 2 * k, fn)
                # paced fillers: ~1 micro-piece per group; when a deadline
                # looms within 8 groups, guarantee at least one pop so a
                # unit's subs drain gradually instead of bursting at the
                # deadline (which would stall the exp stream)
                cq_credit[0] += cq_rate[0]
                if cq and cq[0][0] <= gg + 8:
                    cq_credit[0] = max(cq_credit[0], 1.0)
                while cq and cq_credit[0] >= 1.0:
                    cq_credit[0] -= 1.0
                    cq.pop(0)[2]()
            if it == len(iters) - 1:
                while pend:
                    pend.popleft()()
                # ---- tail: ship the raw hp=1 accumulator (unnormalized
                # per-head products + rowsum row) for the last i-chunk;
                # host divides and applies the small out-projection.  Two
                # casts (DVE + the now-idle ScalarE) + two small DMAs.
                tlA = nrm.tile([128, ICHUNK], F16, tag="tlA", name="tlA")
                tlB = nrm.tile([128, ICHUNK], F16, tag="tlB", name="tlB")
                nc.vector.tensor_copy(tlA[0:65, :], acc[0:65, 0:ICHUNK])
                nc.scalar.copy(tlB[0:65, :], acc[0:65, ICHUNK:])
                nc.sync.dma_start(tl_e[0], tlA[0:65, :])
                nc.scalar.dma_start(tl_e[1], tlB[0:65, :])
                continue

            # ---- normalization: deferred into the next iteration's group-0
            # slot so this iteration's last attn@V flows straight into the
            # next one's scores with filler in between ----
            def make_norm(acc=acc, i=i, hp=hp, it=it):
                def norm():
                    accs = nrm.tile([128, 2 * ICHUNK], F32R, tag="accs",
                                    name=f"accs{i}_{hp}")
                    with nc.allow_low_precision(
                            reason="attn out + softmax denom fp32r"):
                        nc.vector.tensor_copy(accs[0:65, 0:ICHUNK],
                                              acc[0:65, 0:ICHUNK])
                        nc.vector.tensor_copy(accs[0:65, ICHUNK:],
                                              acc[0:65, ICHUNK:])
                    # rowsum [1, 1024]@p64 -> DRAM -> [128, 8] -> recip ->
                    # DRAM -> broadcast-DMA to 128 partitions (no PE/PSUM).
                    # The relayout bounce is essential: DVE RECIPROCAL is
                    # ~6.4ns/elem PER PARTITION (6.5us on [1, 1024] vs 203ns
                    # on [128, 8]).
                    rs_d = drp.tile([2 * ICHUNK], F32R, tag="rs_d",
                                    name=f"rs_d{i}_{hp}")
                    nc.sync.dma_start(rs_d[:], accs[64:65, :])
                    rs128 = nrm.tile([128, 8], F32R, tag="rs128",
                                     name=f"rs128{i}_{hp}")
                    nc.sync.dma_start(
                        rs128[:], rs_d[:].rearrange("(p a) -> p a", p=128))
                    rr128 = nrm.tile([128, 8], F32R, tag="rr128",
                                     name=f"rr128{i}_{hp}")
                    with nc.allow_low_precision(
                            reason="softmax denom recip fp32r"):
                        nc.vector.reciprocal(rr128[:], rs128[:])
                    rr_d = drp.tile([2 * ICHUNK], F32R, tag="rr_d",
                                    name=f"rr_d{i}_{hp}")
                    nc.sync.dma_start(
                        rr_d[:].rearrange("(p a) -> p a", p=128), rr128[:])
                    bcs = nrm.tile([128, 2 * ICHUNK], F32R, tag="bcs",
                                   name=f"bcs{i}_{hp}")
                    nc.sync.dma_start(
                        bcs[:],
                        rr_d[:].unsqueeze(0).broadcast_to([128, 2 * ICHUNK]))
                    on = nrm.tile([128, ICHUNK], BF16, tag="on",
                                  name=f"on{i}_{hp}", bufs=6)
                    with nc.allow_low_precision(
                            reason="attn out normalize bf16"):
                        for h in range(2):
                            nc.vector.tensor_mul(
                                on[64 * h: 64 * h + 64, :],
                                accs[0:64, ICHUNK * h: ICHUNK * (h + 1)],
                                bcs[0:64, ICHUNK * h: ICHUNK * (h + 1)])
                    on_tiles[(i, hp)] = on
                    if hp == 1:
                        # queue i's out-projection into upcoming filler
                        # slots (8 pieces, due progressively).  First piece
                        # +4 groups after the boundary so the normalize's
                        # DRAM round-trip (~3us) completes before the PE
                        # reaches the first outproj matmul in program order.
                        for k, fn in enumerate(
                                outproj_pieces(i, on_tiles[(i, 0)], on,
                                               False)):
                            push((it + 1) * NJT + 4 + 2 * k, fn)
                return norm
            # ride the normalize right behind this iteration's last attn@V
            last_attnv = pend.pop()
            pend.append(lambda f=last_attnv, n=make_norm(): (f(), n()))

        while cq:
            cq.pop(0)[2]()

    nc.compile()
    return nc


def _get_program():
    global _PROGRAM
    if _PROGRAM is None:
        _PROGRAM = _build_program()
    return _PROGRAM


def _prepare_in_maps(x, context, Wq, Wk, Wv, Wo, bo):
    import ml_dtypes
    bf16 = ml_dtypes.bfloat16

    x = np.asarray(x, dtype=np.float32)
    context = np.asarray(context, dtype=np.float32)
    Wq = np.asarray(Wq, dtype=np.float32)
    Wk = np.asarray(Wk, dtype=np.float32)
    Wv = np.asarray(Wv, dtype=np.float32)
    Wo = np.asarray(Wo, dtype=np.float32)
    Wk_s = Wk * np.float32(SCALE)

    def featmaj(a):  # [N, D] -> [NCH, 128, KT, TOKCHUNK] bf16
        t = a.T.reshape(KT, 128, NCH, TOKCHUNK).transpose(2, 1, 0, 3)
        return np.ascontiguousarray(t).astype(bf16)

    xT = [featmaj(x[b]) for b in range(B)]
    cT = [featmaj(context[b]) for b in range(B)]

    in_maps = []
    for c in range(NCORES):
        b, g = c // 4, c % 4
        cs = slice(g * HDC, (g + 1) * HDC)
        in_maps.append({
            "xt": xT[b],
            "ct": cT[b],
            "wq": np.ascontiguousarray(Wq[:, cs]).astype(bf16),
            "wk": np.ascontiguousarray(Wk_s[:, cs]).astype(bf16),
            "wv": np.ascontiguousarray(Wv[:, cs]).astype(bf16),
            "wo": np.ascontiguousarray(Wo[cs, :]).astype(bf16),
        })
    return in_maps


def _gather(results, bo, Wo):
    bo = np.asarray(bo, dtype=np.float32)
    Wo = np.asarray(Wo, dtype=np.float32)
    last = slice(NT - ICHUNK, NT)
    outs = []
    for b in range(B):
        acc = results[4 * b]["out"].astype(np.float64)
        for g in range(1, 4):
            acc += results[4 * b + g]["out"]
        for g in range(4):
            r = results[4 * b + g]
            tl = r["tl"].astype(np.float64)          # [2, 65, ICHUNK]
            for h in range(2):
                attn = tl[h, 0:64] / tl[h, 64]       # [64, ICHUNK]
                row0 = g * HDC + 128 + 64 * h
                wo_h = Wo[row0: row0 + 64]           # [64, D]
                acc[:, last] += wo_h.T @ attn
        outs.append(acc.T.astype(np.float32) + bo)
    return np.stack(outs).reshape(B, N, D)


def kernel(x, context, Wq, Wk, Wv, Wo, bo):
    from concourse.bass_utils import run_bass_kernel_spmd

    in_maps = _prepare_in_maps(x, context, Wq, Wk, Wv, Wo, bo)
    nc = _get_program()
    res = run_bass_kernel_spmd(nc, in_maps, list(range(NCORES)))
    return _gather(res.results, bo, Wo)

